# revision 4
# baseline (speedup 1.0000x reference)
"""Trainium2 Bass kernel for nn_ContrastiveSWM (GNN message passing).

Math (per reference.py):
  B=1024, N=8, D=256, H=512, A=4.  flat = states.reshape(B*N, D)
  Edge list over N-1=7 "virtual" objects with stride 7: flat rows
  [7g, 7g+7) form 1024 independent fully-connected 7-node blocks
  (rows 0..7167); rows 7168..8191 have no edges.
  8 rounds of: e = edgeMLP([flat[row], node[row], flat[col]]);
  agg = segsum(e, row); node = nodeMLP([flat, av, agg]); flat += node.

Key algebraic restructurings (exact, not approximations):
  * edge MLP layer 1 splits: u_i = flat_i@w1a + node_i@w1b (+b1),
    v_j = flat_j@w1c; per-edge preact = u_i + v_j.
  * edge MLP layer 3 + segment-sum commute: agg_i = (sum_j h2_ij)@e_w3
    and e_w3 folds into the node MLP: agg@n_w1g = s@(e_w3@n_w1g).
  * LN mean is LINEAR in h1, so it folds into the layer-2 weights:
    t = z2 - mean = (W2 - rowmean(W2)) @ h1 = W2c @ h1. No mean matmul,
    no mean broadcast-subtract pass. Same for the node LN (nw2c).
  * edge layer 2 runs with SWAPPED operands (lhsT = h1 column-chunk,
    rhs = W2c) so t comes out ROW-major ([128 edge rows, 512 feats]).
    Then LN variance is a per-partition free-dim reduction (one fused
    Square+accum_out ACT pass over PSUM), rstd is a per-partition
    scalar, and h2 = relu(t)*rstd is ONE fused op (ACT Relu with
    scale=rstd, or DVE tensor_scalar mult+max) directly from PSUM.
  * The j-aggregation sums 6 row-major tiles (DVE/Pool adds), and a
    tiny PE transpose (4x [128,128]) restores s to feature-major for
    the node-MLP matmul.

Sharding: data-parallel over the 1024 edge-blocks: core d owns blocks
[128d,128(d+1)) = flat rows [896d, 896(d+1)), plus tail rows
[7168+128d, 7168+128(d+1)).  Weights replicated.  No collectives.

On-device layout is feature-major for matmul inputs (features on
partitions), slot-major columns (col = slot*128 + block) so the
fully-connected gather is contiguous 128-column slices.

Dtypes: residual stream (flat/node) and layer-1 weights float32r;
activations/other weights fp16; PSUM fp32. fp8 was evaluated and
rejected: measured 3.7e-2 absmax-rel (gate 2e-2).
"""

import numpy as np

try:
    import concourse.bass as bass
except ImportError:  # environment fallback
    import sys

    sys.path.insert(0, "/opt/trn_rl_repo")
    import concourse.bass as bass

import concourse.mybir as mybir
import concourse.tile as tile
from concourse import bacc, bass_utils

F32 = mybir.dt.float32
F32R = mybir.dt.float32r
BF16 = mybir.dt.float16  # fp16: same speed class as bf16, 8x finer mantissa
AL = mybir.AluOpType
AF = mybir.ActivationFunctionType

B, N, D, H, A = 1024, 8, 256, 512, 4
M = 8  # cores
NB = 128  # blocks per core
S = N - 1  # 7 slots per block
EC = NB * S  # 896 edge cols per core
TC = (B * N - B * S) // M  # 128 tail cols per core
C = EC + TC  # 1024 cols per core
ROUNDS = N  # 8
EPS = 1e-5
ND = D // 128  # 2
NH = H // 128  # 4
W = S - 1  # 6 j-groups per slot
EW = W * 128  # 768

_CACHE = {}


def _bc(ap2d, n, w):
    """[p, w] -> [p, n, w] broadcast along a new middle dim."""
    return ap2d.unsqueeze(1).to_broadcast((ap2d.shape[0], n, w))


def _force_single_act_set(nc):
    """All activation funcs we use live in one set; strip them from every
    other set so the table-load assigner never thrashes."""
    from concourse import hw_specs

    tables = hw_specs.get_activation_tables(nc.m.arch)
    used = {AF.Copy, AF.Relu, AF.Square, AF.Abs_reciprocal_sqrt, AF.Identity}
    keep = "abs_reciprocal_sqrt_and_small"
    assert used <= tables[keep], (used - tables[keep])
    for name, s in tables.items():
        if name != keep:
            s.difference_update(used)


def build_program():
    nc = bacc.Bacc("TRN2", target_bir_lowering=False, debug=False, num_devices=M)
    _force_single_act_set(nc)

    # ---------------- DRAM I/O ----------------
    xT = nc.dram_tensor("xT", [ND, 128, C], F32R, kind="ExternalInput")
    ohT = nc.dram_tensor("ohT", [A, C], BF16, kind="ExternalInput")
    w1a_d = nc.dram_tensor("w1a", [ND, 128, H], F32R, kind="ExternalInput")
    w1b_d = nc.dram_tensor("w1b", [ND, 128, H], F32R, kind="ExternalInput")
    w1c_d = nc.dram_tensor("w1c", [ND, 128, H], F32R, kind="ExternalInput")
    n1x_d = nc.dram_tensor("n1x", [ND, 128, H], F32R, kind="ExternalInput")
    n1a_d = nc.dram_tensor("n1a", [A, H], BF16, kind="ExternalInput")
    ew2c_d = nc.dram_tensor("ew2c", [NH, 128, H], BF16, kind="ExternalInput")
    wsg_d = nc.dram_tensor("wsg", [NH, 128, H], BF16, kind="ExternalInput")
    nw2c_d = nc.dram_tensor("nw2c", [NH, 128, H], BF16, kind="ExternalInput")
    nw3_d = nc.dram_tensor("nw3", [NH, 128, D], BF16, kind="ExternalInput")
    ident_d = nc.dram_tensor("ident", [128, 128], BF16, kind="ExternalInput")
    yT = nc.dram_tensor("yT", [ND, 128, C], F32R, kind="ExternalOutput")

    with tile.TileContext(nc) as tc:
        with (
            tc.tile_pool(name="persist", bufs=1) as pp,
            tc.tile_pool(name="h1p", bufs=2) as ph1,
            tc.tile_pool(name="h2p", bufs=2) as ph2,
            tc.tile_pool(name="stat", bufs=2) as pst,
            tc.tile_pool(name="srow", bufs=2) as psr,
            tc.tile_pool(name="jsum", bufs=2) as pjs,
            tc.tile_pool(name="noded", bufs=1) as pn,
            tc.tile_pool(name="tmm", bufs=3, space="PSUM") as pt,
            tc.tile_pool(name="mm", bufs=2, space="PSUM") as pm,
            tc.tile_pool(name="trp", bufs=1, space="PSUM") as ptr,
        ):
            # ---------------- persistent SBUF ----------------
            flat_t = pp.tile([128, ND, C], F32R)
            node_t = pp.tile([128, ND, C], F32R)
            u_t = pp.tile([128, NH, EC], BF16)
            v_t = pp.tile([128, NH, EC], BF16)
            s_t = pp.tile([128, NH, C], BF16)
            m1_t = pp.tile([128, NH, C], BF16)
            m2_t = pp.tile([128, NH, C], BF16)
            w1a_t = pp.tile([128, ND, H], F32R)
            w1b_t = pp.tile([128, ND, H], F32R)
            w1c_t = pp.tile([128, ND, H], F32R)
            n1x_t = pp.tile([128, ND, H], F32R)
            n1a_t = pp.tile([128, H], BF16)  # rows [:A] used
            oh_t = pp.tile([128, C], BF16)  # rows [:A] used
            ew2c_t = pp.tile([128, NH, H], BF16)
            wsg_t = pp.tile([128, NH, H], BF16)
            nw2c_t = pp.tile([128, NH, H], BF16)
            nw3_t = pp.tile([128, NH, D], BF16)
            ident_t = pp.tile([128, 128], BF16)
            ones_t = pp.tile([128, 128], BF16)
            eps_t = pp.tile([128, 1], F32)
            scr_act = pp.tile([128, H], BF16)  # dummy Square outputs
            scr_dve = pp.tile([128, H], BF16)

            nc.vector.memset(ones_t[:], 1.0 / H)
            nc.vector.memset(s_t[:, :, EC:C], 0.0)
            nc.vector.memset(eps_t[:], EPS)
            for k in range(ND):
                nc.sync.dma_start(flat_t[:, k, :], xT[k])
                nc.sync.dma_start(w1a_t[:, k, :], w1a_d[k])
                nc.sync.dma_start(w1c_t[:, k, :], w1c_d[k])
            for k in range(ND):
                nc.sync.dma_start(w1b_t[:, k, :], w1b_d[k])
                nc.sync.dma_start(n1x_t[:, k, :], n1x_d[k])
            for k in range(NH):
                nc.sync.dma_start(ew2c_t[:, k, :], ew2c_d[k])
                nc.sync.dma_start(wsg_t[:, k, :], wsg_d[k])
                nc.sync.dma_start(nw2c_t[:, k, :], nw2c_d[k])
                nc.sync.dma_start(nw3_t[:, k, :], nw3_d[k])
            nc.sync.dma_start(n1a_t[:A, :], n1a_d[:])
            nc.sync.dma_start(oh_t[:A, :], ohT[:])
            nc.sync.dma_start(ident_t[:], ident_d[:])

            ECH = [(0, 512), (512, 384)]  # edge-col chunks
            NCH = [(0, 512), (512, 512)]  # node-col chunks (s zero-padded in tail)

            def emit_uv(include_node):
                """u = flat@w1a (+ node@w1b) -> bf16 u_t; v = flat@w1c -> v_t.
                Edge cols only."""
                for c0, cw in ECH:
                    for hp in range(2):
                        up = pm.tile([128, 2, 512], F32, tag="mm")
                        vp = pm.tile([128, 2, 512], F32, tag="mm")
                        for hh in range(2):
                            h = hp * 2 + hh
                            hs = slice(h * 128, (h + 1) * 128)
                            n_acc = ND * (2 if include_node else 1)
                            idx = 0
                            for k in range(ND):
                                nc.tensor.matmul(
                                    up[:, hh, :cw],
                                    lhsT=w1a_t[:, k, hs],
                                    rhs=flat_t[:, k, c0 : c0 + cw],
                                    start=(idx == 0),
                                    stop=(idx == n_acc - 1),
                                )
                                idx += 1
                            if include_node:
                                for k in range(ND):
                                    nc.tensor.matmul(
                                        up[:, hh, :cw],
                                        lhsT=w1b_t[:, k, hs],
                                        rhs=node_t[:, k, c0 : c0 + cw],
                                        start=(idx == 0),
                                        stop=(idx == n_acc - 1),
                                    )
                                    idx += 1
                            for k in range(ND):
                                nc.tensor.matmul(
                                    vp[:, hh, :cw],
                                    lhsT=w1c_t[:, k, hs],
                                    rhs=flat_t[:, k, c0 : c0 + cw],
                                    start=(k == 0),
                                    stop=(k == ND - 1),
                                )
                        nc.scalar.copy(
                            out=u_t[:, hp * 2 : hp * 2 + 2, c0 : c0 + cw],
                            in_=up[:, :, :cw],
                        )
                        nc.scalar.copy(
                            out=v_t[:, hp * 2 : hp * 2 + 2, c0 : c0 + cw],
                            in_=vp[:, :, :cw],
                        )

            def stage_h1(i):
                """h1 = relu(u_i + v_j) for the 6 j-groups, feature-major."""
                h1 = ph1.tile([128, NH, EW], BF16, tag="h1")
                for h in range(NH):
                    ui = u_t[:, h, i * 128 : (i + 1) * 128]
                    if i > 0:
                        nc.vector.tensor_tensor(
                            h1[:, h, 0 : i * 128].rearrange("p (j c) -> p j c", c=128),
                            v_t[:, h, 0 : i * 128].rearrange("p (j c) -> p j c", c=128),
                            _bc(ui, i, 128),
                            AL.add,
                        )
                    if i < W:
                        nj = W - i
                        nc.vector.tensor_tensor(
                            h1[:, h, i * 128 : EW].rearrange("p (j c) -> p j c", c=128),
                            v_t[:, h, (i + 1) * 128 : EC].rearrange(
                                "p (j c) -> p j c", c=128
                            ),
                            _bc(ui, nj, 128),
                            AL.add,
                        )
                nc.vector.tensor_scalar_max(h1[:, 0:2, :], h1[:, 0:2, :], 0.0)
                nc.vector.tensor_scalar_max(h1[:, 2:4, :], h1[:, 2:4, :], 0.0)
                return h1

            # engine split for the per-chunk LN passes (tunable)
            A_ON_ACT = {0, 1, 2, 3, 4, 5}  # Square+accum on ACT for these chunks
            B_ON_ACT = set()  # relu*rstd on ACT for these chunks

            def stage_t(i, h1):
                """Row-major t = W2c@h1 per j-chunk; fused var + h2."""
                h2 = ph2.tile([128, W, H], BF16, tag="h2")
                ssq = pst.tile([128, W], F32, tag="ssq")
                rstd = pst.tile([128, W], F32, tag="rstd")
                for c in range(W):
                    t_ps = pt.tile([128, H], F32, tag="t")
                    for k in range(NH):
                        nc.tensor.matmul(
                            t_ps[:, :],
                            lhsT=h1[:, k, c * 128 : (c + 1) * 128],
                            rhs=ew2c_t[:, k, :],
                            start=(k == 0),
                            stop=(k == NH - 1),
                        )
                    if c in A_ON_ACT:
                        nc.scalar.activation(
                            scr_act[:],
                            t_ps[:, :],
                            AF.Square,
                            scale=0.125,
                            accum_out=ssq[:, c : c + 1],
                        )
                    else:
                        nc.vector.tensor_tensor_reduce(
                            out=scr_dve[:],
                            in0=t_ps[:, :],
                            in1=t_ps[:, :],
                            scale=1.0 / 64.0,
                            scalar=0.0,
                            op0=AL.mult,
                            op1=AL.add,
                            accum_out=ssq[:, c : c + 1],
                        )
                    # ssq holds sum(t^2)/64; var = 64*ssq/H
                    nc.scalar.activation(
                        rstd[:, c : c + 1],
                        ssq[:, c : c + 1],
                        AF.Abs_reciprocal_sqrt,
                        bias=eps_t[:, 0:1],
                        scale=64.0 / H,
                    )
                    if c in B_ON_ACT:
                        nc.scalar.activation(
                            h2[:, c, :], t_ps[:, :], AF.Relu, scale=rstd[:, c : c + 1]
                        )
                    else:
                        nc.vector.tensor_scalar(
                            h2[:, c, :],
                            t_ps[:, :],
                            rstd[:, c : c + 1],
                            0.0,
                            AL.mult,
                            AL.max,
                        )
                return h2

            def stage_s(i, h2):
                """s_row = sum_j h2_j (row-major [128 blocks, H])."""
                s_row = psr.tile([128, H], BF16, tag="srow")
                p01 = pjs.tile([128, H], BF16, tag="p01")
                p23 = pjs.tile([128, H], BF16, tag="p23")
                nc.vector.tensor_add(p01[:, :], h2[:, 0, :], h2[:, 1, :])
                nc.gpsimd.tensor_add(p23[:, :], h2[:, 2, :], h2[:, 3, :])
                nc.vector.tensor_add(s_row[:, :], h2[:, 4, :], h2[:, 5, :])
                nc.vector.tensor_add(p01[:, :], p01[:, :], p23[:, :])
                nc.vector.tensor_add(s_row[:, :], s_row[:, :], p01[:, :])
                return s_row

            def stage_tr(i, s_row):
                """Transpose s_row back to feature-major into s_t slot i."""
                tr = ptr.tile([128, NH, 128], BF16, tag="tr")
                for c in range(NH):
                    nc.tensor.matmul(
                        tr[:, c, :],
                        lhsT=s_row[:, c * 128 : (c + 1) * 128],
                        rhs=ident_t[:],
                        is_transpose=True,
                    )
                nc.vector.tensor_copy(
                    out=s_t[:, :, i * 128 : (i + 1) * 128], in_=tr[:, :, :]
                )

            def emit_m1(c, r):
                # m1 chunk = relu(flat@n1x + s@wsg (+ oh@n1a at r=0))
                c0, cw = NCH[c]
                for hp in range(2):
                    m1p = pm.tile([128, 2, 512], F32, tag="mm")
                    for hh in range(2):
                        h = hp * 2 + hh
                        hs = slice(h * 128, (h + 1) * 128)
                        n_acc = ND + NH + (1 if r == 0 else 0)
                        idx = 0
                        for k in range(ND):
                            nc.tensor.matmul(
                                m1p[:, hh, :cw],
                                lhsT=n1x_t[:, k, hs],
                                rhs=flat_t[:, k, c0 : c0 + cw],
                                start=(idx == 0),
                                stop=(idx == n_acc - 1),
                            )
                            idx += 1
                        for k in range(NH):
                            nc.tensor.matmul(
                                m1p[:, hh, :cw],
                                lhsT=wsg_t[:, k, hs],
                                rhs=s_t[:, k, c0 : c0 + cw],
                                start=(idx == 0),
                                stop=(idx == n_acc - 1),
                            )
                            idx += 1
                        if r == 0:
                            nc.tensor.matmul(
                                m1p[:, hh, :cw],
                                lhsT=n1a_t[:A, hs],
                                rhs=oh_t[:A, c0 : c0 + cw],
                                start=(idx == 0),
                                stop=(idx == n_acc - 1),
                            )
                            idx += 1
                    nc.scalar.activation(
                        m1_t[:, hp * 2 : hp * 2 + 2, c0 : c0 + cw],
                        m1p[:, :, :cw],
                        AF.Relu,
                    )

            def emit_node(r):
                t_n = pn.tile([128, NH, C], BF16, tag="t_n")
                sq_n = pn.tile([128, NH, C], BF16, tag="sq_n")
                rstd_n = pn.tile([128, C], BF16, tag="rstd_n")

                def node_a(c):
                    # t_n = nw2c @ m1 (feature-major; mean already folded in)
                    c0, cw = NCH[c]
                    cs = slice(c0, c0 + cw)
                    for hp in range(2):
                        zp = pm.tile([128, 2, 512], F32, tag="mm")
                        for hh in range(2):
                            h = hp * 2 + hh
                            hs = slice(h * 128, (h + 1) * 128)
                            for k in range(NH):
                                nc.tensor.matmul(
                                    zp[:, hh, :cw],
                                    lhsT=nw2c_t[:, k, hs],
                                    rhs=m1_t[:, k, cs],
                                    start=(k == 0),
                                    stop=(k == NH - 1),
                                )
                        nc.scalar.copy(
                            out=t_n[:, hp * 2 : hp * 2 + 2, cs], in_=zp[:, :, :cw]
                        )

                def node_b(c):
                    c0, cw = NCH[c]
                    cs = slice(c0, c0 + cw)
                    nc.vector.tensor_tensor(
                        sq_n[:, :, cs], t_n[:, :, cs], t_n[:, :, cs], AL.mult
                    )
                    var_ps = pt.tile([128, H], F32, tag="t")
                    for k in range(NH):
                        nc.tensor.matmul(
                            var_ps[:, :cw],
                            lhsT=ones_t[:],
                            rhs=sq_n[:, k, cs],
                            start=(k == 0),
                            stop=(k == NH - 1),
                        )
                    nc.scalar.activation(
                        rstd_n[:, cs],
                        var_ps[:, :cw],
                        AF.Abs_reciprocal_sqrt,
                        bias=eps_t[:, 0:1],
                        scale=1.0,
                    )
                    nc.vector.tensor_tensor(
                        m2_t[:, :, cs], t_n[:, :, cs], _bc(rstd_n[:, cs], NH, cw), AL.mult
                    )
                    nc.vector.tensor_scalar_max(m2_t[:, :, cs], m2_t[:, :, cs], 0.0)

                def node_c(c):
                    c0, cw = NCH[c]
                    cs = slice(c0, c0 + cw)
                    np_ = pm.tile([128, 2, 512], F32, tag="mm")
                    for dd in range(ND):
                        ds_ = slice(dd * 128, (dd + 1) * 128)
                        for k in range(NH):
                            nc.tensor.matmul(
                                np_[:, dd, :cw],
                                lhsT=nw3_t[:, k, ds_],
                                rhs=m2_t[:, k, cs],
                                start=(k == 0),
                                stop=(k == NH - 1),
                            )
                    nc.scalar.copy(out=node_t[:, :, cs], in_=np_[:, :, :cw])
                    for k in range(ND):
                        nc.vector.tensor_tensor(
                            flat_t[:, k, cs], flat_t[:, k, cs], node_t[:, k, cs], AL.add
                        )
                    if r < ROUNDS - 1:
                        cw_uv = min(c0 + cw, EC) - c0
                        for hp in range(2):
                            up = pm.tile([128, 2, 512], F32, tag="mm")
                            vp = pm.tile([128, 2, 512], F32, tag="mm")
                            for hh in range(2):
                                h = hp * 2 + hh
                                hs = slice(h * 128, (h + 1) * 128)
                                for k in range(ND):
                                    nc.tensor.matmul(
                                        up[:, hh, :cw_uv],
                                        lhsT=w1a_t[:, k, hs],
                                        rhs=flat_t[:, k, c0 : c0 + cw_uv],
                                        start=(k == 0),
                                        stop=False,
                                    )
                                for k in range(ND):
                                    nc.tensor.matmul(
                                        up[:, hh, :cw_uv],
                                        lhsT=w1b_t[:, k, hs],
                                        rhs=node_t[:, k, c0 : c0 + cw_uv],
                                        start=False,
                                        stop=(k == ND - 1),
                                    )
                                for k in range(ND):
                                    nc.tensor.matmul(
                                        vp[:, hh, :cw_uv],
                                        lhsT=w1c_t[:, k, hs],
                                        rhs=flat_t[:, k, c0 : c0 + cw_uv],
                                        start=(k == 0),
                                        stop=(k == ND - 1),
                                    )
                            nc.scalar.copy(
                                out=v_t[:, hp * 2 : hp * 2 + 2, c0 : c0 + cw_uv],
                                in_=vp[:, :, :cw_uv],
                            )
                            nc.scalar.copy(
                                out=u_t[:, hp * 2 : hp * 2 + 2, c0 : c0 + cw_uv],
                                in_=up[:, :, :cw_uv],
                            )

                node_a(0)
                node_a(1)
                node_b(0)
                node_b(1)
                node_c(0)
                node_c(1)

            # ---------------- the 8 rounds ----------------
            emit_uv(include_node=False)
            for r in range(ROUNDS):
                pending = None  # (slot, s_row) awaiting transpose
                for i in range(S):
                    h1 = stage_h1(i)
                    h2 = stage_t(i, h1)
                    if pending is not None:
                        stage_tr(*pending)
                    pending = (i, stage_s(i, h2))
                    if i == 6:
                        emit_m1(0, r)  # s slots 0-3 ready well before this
                stage_tr(*pending)
                emit_m1(1, r)
                emit_node(r)
                if r < ROUNDS - 1:
                    emit_uv(include_node=True)

            for k in range(ND):
                nc.sync.dma_start(yT[k], flat_t[:, k, :])

    nc.compile()
    return nc


# ---------------------------------------------------------------------------
# Host side
# ---------------------------------------------------------------------------


def _host_prep(inputs):
    """Build per-core input maps.  Returns (in_maps, perm) where perm maps
    device column order back to global flat-row order."""
    states = np.asarray(inputs["states"], np.float32).reshape(B * N, D)
    action = np.asarray(inputs["action"]).astype(np.int64)

    e_w1 = np.asarray(inputs["e_w1"], np.float32)
    e_w2 = np.asarray(inputs["e_w2"], np.float32)
    e_w3 = np.asarray(inputs["e_w3"], np.float32)
    n_w1 = np.asarray(inputs["n_w1"], np.float32)
    n_w2 = np.asarray(inputs["n_w2"], np.float32)
    n_w3 = np.asarray(inputs["n_w3"], np.float32)

    w1a, w1b, w1c = e_w1[0:D], e_w1[D : 2 * D], e_w1[2 * D : 3 * D]
    n1x = n_w1[0:D]
    n1a = n_w1[D : D + A]
    n1g = n_w1[D + A :]
    wsg = e_w3 @ n1g  # [H, H]

    # LN mean folded into centered layer-2 weights
    ew2c = e_w2 - e_w2.mean(1, keepdims=True)
    nw2c = n_w2 - n_w2.mean(1, keepdims=True)

    onehot = np.zeros((B, A), np.float32)
    onehot[np.arange(B), action] = 1.0

    # device column -> global flat row, per core
    perms = []
    for d in range(M):
        edge_rows = np.empty(EC, np.int64)
        for s in range(S):
            for b in range(NB):
                edge_rows[s * NB + b] = 896 * d + 7 * b + s
        tail_rows = np.arange(B * S + TC * d, B * S + TC * (d + 1), dtype=np.int64)
        perms.append(np.concatenate([edge_rows, tail_rows]))

    def kt(w, nk):  # [K, F] -> [nk, 128, F]
        return np.ascontiguousarray(w.reshape(nk, 128, -1))

    bf = np.float16
    common = {
        "w1a": kt(w1a, ND),
        "w1b": kt(w1b, ND),
        "w1c": kt(w1c, ND),
        "n1x": kt(n1x, ND),
        "n1a": np.ascontiguousarray(n1a).astype(bf),
        "ew2c": kt(ew2c, NH).astype(bf),
        "wsg": kt(wsg, NH).astype(bf),
        "nw2c": kt(nw2c, NH).astype(bf),
        "nw3": kt(n_w3, NH).astype(bf),
        "ident": np.eye(128, dtype=bf),
    }

    in_maps = []
    for d in range(M):
        rows = perms[d]
        xT_d = np.ascontiguousarray(states[rows].T.reshape(ND, 128, C))
        oh_d = np.ascontiguousarray(onehot[rows // N].T).astype(bf)  # [A, C]
        in_maps.append({"xT": xT_d, "ohT": oh_d, **common})
    return in_maps, perms


def _check_fast_path(inputs):
    z = lambda k: np.allclose(np.asarray(inputs[k]), 0.0)
    o = lambda k: np.allclose(np.asarray(inputs[k]), 1.0)
    return (
        z("e_b1") and z("e_b2") and z("e_bn") and z("e_b3")
        and z("n_b1") and z("n_b2") and z("n_bn") and z("n_b3")
        and o("e_g") and o("n_g")
    )


def _numpy_fallback(inputs):
    """Exact NumPy port of reference.py (used only if the fast-path
    assumptions about biases/LN-affine do not hold)."""
    f32 = np.float32
    states = np.asarray(inputs["states"], f32)
    action = np.asarray(inputs["action"]).astype(np.int64)
    g = {k: np.asarray(v, f32) for k, v in inputs.items() if k not in ("states", "action")}

    def ln(x, ga, be):
        m = x.mean(-1, keepdims=True)
        v = x.var(-1, keepdims=True)
        return (x - m) / np.sqrt(v + EPS) * ga + be

    def mlp(x, w1, b1, w2, b2, ga, bn, w3, b3):
        h = np.maximum(x @ w1 + b1, 0)
        h = np.maximum(ln(h @ w2 + b2, ga, bn), 0)
        return h @ w3 + b3

    eP = (g["e_w1"], g["e_b1"], g["e_w2"], g["e_b2"], g["e_g"], g["e_bn"], g["e_w3"], g["e_b3"])
    nP = (g["n_w1"], g["n_b1"], g["n_w2"], g["n_b2"], g["n_g"], g["n_bn"], g["n_w3"], g["n_b3"])
    flat = states.reshape(-1, D)
    pairs = np.array([(i, j) for i in range(S) for j in range(S) if i != j], np.int64)
    off = (np.arange(B, dtype=np.int64) * S)[:, None]
    row = (pairs[:, 0][None, :] + off).reshape(-1)
    col = (pairs[:, 1][None, :] + off).reshape(-1)
    E = row.shape[0]
    onehot = np.zeros((B, A), f32)
    onehot[np.arange(B), action] = 1.0
    av = np.repeat(onehot, N, axis=0)

    def seg_sum(e):
        agg = np.zeros((B * N, H), f32)
        np.add.at(agg, row, e)
        return agg

    e = mlp(np.concatenate([flat[row], np.zeros((E, D), f32), flat[col]], 1), *eP)
    node = mlp(np.concatenate([flat, av, seg_sum(e)], 1), *nP)
    flat = flat + node
    av0 = np.zeros_like(av)
    for _ in range(N - 1):
        e = mlp(np.concatenate([flat[row], node[row], flat[col]], 1), *eP)
        node = mlp(np.concatenate([flat, av0, seg_sum(e)], 1), *nP)
        flat = flat + node
    return flat.reshape(B, N, D).astype(np.float32)


def get_program():
    if "nc" not in _CACHE:
        _CACHE["nc"] = build_program()
    return _CACHE["nc"]


def kernel(**inputs):
    if not _check_fast_path(inputs):
        return _numpy_fallback(inputs)

    nc = get_program()
    in_maps, perms = _host_prep(inputs)
    res = bass_utils.run_bass_kernel_spmd(nc, in_maps, core_ids=list(range(M)))
    _CACHE["last_results"] = res

    out = np.empty((B * N, D), np.float32)
    for d in range(M):
        yT = res.results[d]["yT"].reshape(D, C)  # [D, C]
        out[perms[d]] = yT.T
    return out.reshape(B, N, D)


if __name__ == "__main__":
    rng = np.random.default_rng(0)
    print("building program...")
    nc = get_program()
    print("built.")


# revision 6
# speedup vs baseline: 1.1644x; 1.1644x over previous
"""Trainium2 Bass kernel for nn_ContrastiveSWM (GNN message passing).

Math (per reference.py):
  B=1024, N=8, D=256, H=512, A=4.  flat = states.reshape(B*N, D)
  Edge list over N-1=7 "virtual" objects with stride 7: flat rows
  [7g, 7g+7) form 1024 independent fully-connected 7-node blocks
  (rows 0..7167); rows 7168..8191 have no edges.
  8 rounds of: e = edgeMLP([flat[row], node[row], flat[col]]);
  agg = segsum(e, row); node = nodeMLP([flat, av, agg]); flat += node.

Key algebraic restructurings (exact, not approximations):
  * edge MLP layer 1 splits: u_i = flat_i@w1a + node_i@w1b (+b1),
    v_j = flat_j@w1c; per-edge preact = u_i + v_j.
  * edge MLP layer 3 + segment-sum commute: agg_i = (sum_j h2_ij)@e_w3
    and e_w3 folds into the node MLP: agg@n_w1g = s@(e_w3@n_w1g).
  * LN mean is LINEAR in h1, so it folds into the layer-2 weights:
    t = z2 - mean = (W2 - rowmean(W2)) @ h1 = W2c @ h1. No mean matmul,
    no mean broadcast-subtract pass. Same for the node LN (nw2c).
  * edge layer 2 runs with SWAPPED operands (lhsT = h1 column-chunk,
    rhs = W2c) so t comes out ROW-major ([128 edge rows, 512 feats]).
    Then LN variance is a per-partition free-dim reduction (one fused
    Square+accum_out ACT pass over PSUM), rstd is a per-partition
    scalar, and h2 = relu(t)*rstd is ONE fused op (ACT Relu with
    scale=rstd, or DVE tensor_scalar mult+max) directly from PSUM.
  * The j-aggregation sums 6 row-major tiles (DVE/Pool adds), and a
    tiny PE transpose (4x [128,128]) restores s to feature-major for
    the node-MLP matmul.

Sharding: data-parallel over the 1024 edge-blocks: core d owns blocks
[128d,128(d+1)) = flat rows [896d, 896(d+1)), plus tail rows
[7168+128d, 7168+128(d+1)).  Weights replicated.  No collectives.

On-device layout is feature-major for matmul inputs (features on
partitions), slot-major columns (col = slot*128 + block) so the
fully-connected gather is contiguous 128-column slices.

Dtypes: residual stream (flat/node) and layer-1 weights float32r;
activations/other weights fp16; PSUM fp32. fp8 was evaluated and
rejected: measured 3.7e-2 absmax-rel (gate 2e-2).
"""

import numpy as np

try:
    import concourse.bass as bass
except ImportError:  # environment fallback
    import sys

    sys.path.insert(0, "/opt/trn_rl_repo")
    import concourse.bass as bass

import concourse.mybir as mybir
import concourse.tile as tile
from concourse import bacc, bass_utils

F32 = mybir.dt.float32
F32R = mybir.dt.float32r
BF16 = mybir.dt.float16  # fp16: same speed class as bf16, 8x finer mantissa
AL = mybir.AluOpType
AF = mybir.ActivationFunctionType

B, N, D, H, A = 1024, 8, 256, 512, 4
M = 8  # cores
NB = 128  # blocks per core
S = N - 1  # 7 slots per block
EC = NB * S  # 896 edge cols per core
TC = (B * N - B * S) // M  # 128 tail cols per core
C = EC + TC  # 1024 cols per core
ROUNDS = N  # 8
EPS = 1e-5
ND = D // 128  # 2
NH = H // 128  # 4
W = S - 1  # 6 j-groups per slot
EW = W * 128  # 768

_CACHE = {}


def _bc(ap2d, n, w):
    """[p, w] -> [p, n, w] broadcast along a new middle dim."""
    return ap2d.unsqueeze(1).to_broadcast((ap2d.shape[0], n, w))


def _force_single_act_set(nc):
    """All activation funcs we use live in one set; strip them from every
    other set so the table-load assigner never thrashes."""
    from concourse import hw_specs

    tables = hw_specs.get_activation_tables(nc.m.arch)
    used = {AF.Copy, AF.Relu, AF.Square, AF.Abs_reciprocal_sqrt, AF.Identity}
    keep = "abs_reciprocal_sqrt_and_small"
    assert used <= tables[keep], (used - tables[keep])
    for name, s in tables.items():
        if name != keep:
            s.difference_update(used)


def build_program():
    nc = bacc.Bacc("TRN2", target_bir_lowering=False, debug=False, num_devices=M)
    _force_single_act_set(nc)

    # ---------------- DRAM I/O ----------------
    xT = nc.dram_tensor("xT", [ND, 128, C], F32R, kind="ExternalInput")
    ohT = nc.dram_tensor("ohT", [A, C], BF16, kind="ExternalInput")
    w1a_d = nc.dram_tensor("w1a", [ND, 128, H], F32R, kind="ExternalInput")
    w1b_d = nc.dram_tensor("w1b", [ND, 128, H], F32R, kind="ExternalInput")
    w1c_d = nc.dram_tensor("w1c", [ND, 128, H], F32R, kind="ExternalInput")
    n1x_d = nc.dram_tensor("n1x", [ND, 128, H], F32R, kind="ExternalInput")
    n1a_d = nc.dram_tensor("n1a", [A, H], BF16, kind="ExternalInput")
    ew2c_d = nc.dram_tensor("ew2c", [NH, 128, H], BF16, kind="ExternalInput")
    wsg_d = nc.dram_tensor("wsg", [NH, 128, H], BF16, kind="ExternalInput")
    nw2c_d = nc.dram_tensor("nw2c", [NH, 128, H], BF16, kind="ExternalInput")
    nw3_d = nc.dram_tensor("nw3", [NH, 128, D], BF16, kind="ExternalInput")
    ident_d = nc.dram_tensor("ident", [128, 128], BF16, kind="ExternalInput")
    yT = nc.dram_tensor("yT", [ND, 128, C], F32R, kind="ExternalOutput")

    with tile.TileContext(nc) as tc:
        with (
            tc.tile_pool(name="persist", bufs=1) as pp,
            tc.tile_pool(name="h1p", bufs=2) as ph1,
            tc.tile_pool(name="h2p", bufs=2) as ph2,
            tc.tile_pool(name="stat", bufs=2) as pst,
            tc.tile_pool(name="srow", bufs=2) as psr,
            tc.tile_pool(name="jsum", bufs=2) as pjs,
            tc.tile_pool(name="noded", bufs=1) as pn,
            tc.tile_pool(name="tmm", bufs=3, space="PSUM") as pt,
            tc.tile_pool(name="mm", bufs=2, space="PSUM") as pm,
            tc.tile_pool(name="trp", bufs=1, space="PSUM") as ptr,
        ):
            # ---------------- persistent SBUF ----------------
            flat_t = pp.tile([128, ND, C], F32R)
            node_t = pp.tile([128, ND, C], F32R)
            u_t = pp.tile([128, NH, EC], BF16)
            v_t = pp.tile([128, NH, EC], BF16)
            s_t = pp.tile([128, NH, C], BF16)
            m1_t = pp.tile([128, NH, C], BF16)
            m2_t = pp.tile([128, NH, C], BF16)
            w1a_t = pp.tile([128, ND, H], F32R)
            w1b_t = pp.tile([128, ND, H], F32R)
            w1c_t = pp.tile([128, ND, H], F32R)
            n1x_t = pp.tile([128, ND, H], F32R)
            n1a_t = pp.tile([128, H], BF16)  # rows [:A] used
            oh_t = pp.tile([128, C], BF16)  # rows [:A] used
            ew2c_t = pp.tile([128, NH, H], BF16)
            wsg_t = pp.tile([128, NH, H], BF16)
            nw2c_t = pp.tile([128, NH, H], BF16)
            nw3_t = pp.tile([128, NH, D], BF16)
            ident_t = pp.tile([128, 128], BF16)
            ones_t = pp.tile([128, 128], BF16)
            eps_t = pp.tile([128, 1], F32)
            scr_act = pp.tile([128, H], BF16)  # dummy Square outputs
            scr_dve = pp.tile([128, H], BF16)

            nc.vector.memset(ones_t[:], 1.0 / H)
            nc.vector.memset(s_t[:, :, EC:C], 0.0)
            nc.vector.memset(eps_t[:], EPS)
            for k in range(ND):
                nc.sync.dma_start(flat_t[:, k, :], xT[k])
                nc.sync.dma_start(w1a_t[:, k, :], w1a_d[k])
                nc.sync.dma_start(w1c_t[:, k, :], w1c_d[k])
            for k in range(ND):
                nc.sync.dma_start(w1b_t[:, k, :], w1b_d[k])
                nc.sync.dma_start(n1x_t[:, k, :], n1x_d[k])
            for k in range(NH):
                nc.sync.dma_start(ew2c_t[:, k, :], ew2c_d[k])
                nc.sync.dma_start(wsg_t[:, k, :], wsg_d[k])
                nc.sync.dma_start(nw2c_t[:, k, :], nw2c_d[k])
                nc.sync.dma_start(nw3_t[:, k, :], nw3_d[k])
            nc.sync.dma_start(n1a_t[:A, :], n1a_d[:])
            nc.sync.dma_start(oh_t[:A, :], ohT[:])
            nc.sync.dma_start(ident_t[:], ident_d[:])

            ECH = [(0, 512), (512, 384)]  # edge-col chunks
            NCH = [(0, 512), (512, 512)]  # node-col chunks (s zero-padded in tail)

            def emit_uv(include_node):
                """u = flat@w1a (+ node@w1b) -> bf16 u_t; v = flat@w1c -> v_t.
                Edge cols only."""
                for c0, cw in ECH:
                    for hp in range(2):
                        up = pm.tile([128, 2, 512], F32, tag="mm")
                        vp = pm.tile([128, 2, 512], F32, tag="mm")
                        for hh in range(2):
                            h = hp * 2 + hh
                            hs = slice(h * 128, (h + 1) * 128)
                            n_acc = ND * (2 if include_node else 1)
                            idx = 0
                            for k in range(ND):
                                nc.tensor.matmul(
                                    up[:, hh, :cw],
                                    lhsT=w1a_t[:, k, hs],
                                    rhs=flat_t[:, k, c0 : c0 + cw],
                                    start=(idx == 0),
                                    stop=(idx == n_acc - 1),
                                )
                                idx += 1
                            if include_node:
                                for k in range(ND):
                                    nc.tensor.matmul(
                                        up[:, hh, :cw],
                                        lhsT=w1b_t[:, k, hs],
                                        rhs=node_t[:, k, c0 : c0 + cw],
                                        start=(idx == 0),
                                        stop=(idx == n_acc - 1),
                                    )
                                    idx += 1
                            for k in range(ND):
                                nc.tensor.matmul(
                                    vp[:, hh, :cw],
                                    lhsT=w1c_t[:, k, hs],
                                    rhs=flat_t[:, k, c0 : c0 + cw],
                                    start=(k == 0),
                                    stop=(k == ND - 1),
                                )
                        nc.scalar.copy(
                            out=u_t[:, hp * 2 : hp * 2 + 2, c0 : c0 + cw],
                            in_=up[:, :, :cw],
                        )
                        nc.scalar.copy(
                            out=v_t[:, hp * 2 : hp * 2 + 2, c0 : c0 + cw],
                            in_=vp[:, :, :cw],
                        )

            def stage_h1(i):
                """h1 = relu(u_i + v_j) for the 6 j-groups, feature-major."""
                h1 = ph1.tile([128, NH, EW], BF16, tag="h1")
                for h in range(NH):
                    ui = u_t[:, h, i * 128 : (i + 1) * 128]
                    if i > 0:
                        nc.vector.tensor_tensor(
                            h1[:, h, 0 : i * 128].rearrange("p (j c) -> p j c", c=128),
                            v_t[:, h, 0 : i * 128].rearrange("p (j c) -> p j c", c=128),
                            _bc(ui, i, 128),
                            AL.add,
                        )
                    if i < W:
                        nj = W - i
                        nc.vector.tensor_tensor(
                            h1[:, h, i * 128 : EW].rearrange("p (j c) -> p j c", c=128),
                            v_t[:, h, (i + 1) * 128 : EC].rearrange(
                                "p (j c) -> p j c", c=128
                            ),
                            _bc(ui, nj, 128),
                            AL.add,
                        )
                nc.vector.tensor_scalar_max(h1[:, 0:2, :], h1[:, 0:2, :], 0.0)
                nc.vector.tensor_scalar_max(h1[:, 2:4, :], h1[:, 2:4, :], 0.0)
                return h1

            # engine split for the per-chunk LN passes (tunable)
            A_ON_ACT = {0, 1, 2, 3, 4, 5}  # Square+accum on ACT for these chunks
            B_ON_ACT = set()  # relu*rstd on ACT for these chunks

            def stage_t(i, h1):
                """Row-major t = W2c@h1 per j-chunk; fused var + h2."""
                h2 = ph2.tile([128, W, H], BF16, tag="h2")
                ssq = pst.tile([128, W], F32, tag="ssq")
                rstd = pst.tile([128, W], F32, tag="rstd")
                for c in range(W):
                    t_ps = pt.tile([128, H], F32, tag="t")
                    for k in range(NH):
                        nc.tensor.matmul(
                            t_ps[:, :],
                            lhsT=h1[:, k, c * 128 : (c + 1) * 128],
                            rhs=ew2c_t[:, k, :],
                            start=(k == 0),
                            stop=(k == NH - 1),
                        )
                    if c in A_ON_ACT:
                        nc.scalar.activation(
                            scr_act[:],
                            t_ps[:, :],
                            AF.Square,
                            scale=0.125,
                            accum_out=ssq[:, c : c + 1],
                        )
                    else:
                        nc.vector.tensor_tensor_reduce(
                            out=scr_dve[:],
                            in0=t_ps[:, :],
                            in1=t_ps[:, :],
                            scale=1.0 / 64.0,
                            scalar=0.0,
                            op0=AL.mult,
                            op1=AL.add,
                            accum_out=ssq[:, c : c + 1],
                        )
                    # ssq holds sum(t^2)/64; var = 64*ssq/H
                    nc.scalar.activation(
                        rstd[:, c : c + 1],
                        ssq[:, c : c + 1],
                        AF.Abs_reciprocal_sqrt,
                        bias=eps_t[:, 0:1],
                        scale=64.0 / H,
                    )
                    if c in B_ON_ACT:
                        nc.scalar.activation(
                            h2[:, c, :], t_ps[:, :], AF.Relu, scale=rstd[:, c : c + 1]
                        )
                    else:
                        nc.vector.tensor_scalar(
                            h2[:, c, :],
                            t_ps[:, :],
                            rstd[:, c : c + 1],
                            0.0,
                            AL.mult,
                            AL.max,
                        )
                return h2

            def stage_s(i, h2):
                """s_row = sum_j h2_j (row-major [128 blocks, H])."""
                s_row = psr.tile([128, H], BF16, tag="srow")
                p01 = pjs.tile([128, H], BF16, tag="p01")
                p23 = pjs.tile([128, H], BF16, tag="p23")
                nc.vector.tensor_add(p01[:, :], h2[:, 0, :], h2[:, 1, :])
                nc.gpsimd.tensor_add(p23[:, :], h2[:, 2, :], h2[:, 3, :])
                nc.vector.tensor_add(s_row[:, :], h2[:, 4, :], h2[:, 5, :])
                nc.vector.tensor_add(p01[:, :], p01[:, :], p23[:, :])
                nc.vector.tensor_add(s_row[:, :], s_row[:, :], p01[:, :])
                return s_row

            def stage_tr(i, s_row):
                """Transpose s_row back to feature-major into s_t slot i."""
                tr = ptr.tile([128, NH, 128], BF16, tag="tr")
                for c in range(NH):
                    nc.tensor.matmul(
                        tr[:, c, :],
                        lhsT=s_row[:, c * 128 : (c + 1) * 128],
                        rhs=ident_t[:],
                        is_transpose=True,
                    )
                nc.vector.tensor_copy(
                    out=s_t[:, :, i * 128 : (i + 1) * 128], in_=tr[:, :, :]
                )

            def emit_m1(c, r):
                # m1 chunk = relu(flat@n1x + s@wsg (+ oh@n1a at r=0))
                c0, cw = NCH[c]
                for hp in range(2):
                    m1p = pm.tile([128, 2, 512], F32, tag="mm")
                    for hh in range(2):
                        h = hp * 2 + hh
                        hs = slice(h * 128, (h + 1) * 128)
                        n_acc = ND + NH + (1 if r == 0 else 0)
                        idx = 0
                        for k in range(ND):
                            nc.tensor.matmul(
                                m1p[:, hh, :cw],
                                lhsT=n1x_t[:, k, hs],
                                rhs=flat_t[:, k, c0 : c0 + cw],
                                start=(idx == 0),
                                stop=(idx == n_acc - 1),
                            )
                            idx += 1
                        for k in range(NH):
                            nc.tensor.matmul(
                                m1p[:, hh, :cw],
                                lhsT=wsg_t[:, k, hs],
                                rhs=s_t[:, k, c0 : c0 + cw],
                                start=(idx == 0),
                                stop=(idx == n_acc - 1),
                            )
                            idx += 1
                        if r == 0:
                            nc.tensor.matmul(
                                m1p[:, hh, :cw],
                                lhsT=n1a_t[:A, hs],
                                rhs=oh_t[:A, c0 : c0 + cw],
                                start=(idx == 0),
                                stop=(idx == n_acc - 1),
                            )
                            idx += 1
                    nc.scalar.activation(
                        m1_t[:, hp * 2 : hp * 2 + 2, c0 : c0 + cw],
                        m1p[:, :, :cw],
                        AF.Relu,
                    )

            def emit_node(r):
                t_n = pn.tile([128, NH, C], BF16, tag="t_n")
                sq_n = pn.tile([128, NH, C], BF16, tag="sq_n")
                rstd_n = pn.tile([128, C], BF16, tag="rstd_n")

                def node_a(c):
                    # t_n = nw2c @ m1 (feature-major; mean already folded in)
                    c0, cw = NCH[c]
                    cs = slice(c0, c0 + cw)
                    for hp in range(2):
                        zp = pm.tile([128, 2, 512], F32, tag="mm")
                        for hh in range(2):
                            h = hp * 2 + hh
                            hs = slice(h * 128, (h + 1) * 128)
                            for k in range(NH):
                                nc.tensor.matmul(
                                    zp[:, hh, :cw],
                                    lhsT=nw2c_t[:, k, hs],
                                    rhs=m1_t[:, k, cs],
                                    start=(k == 0),
                                    stop=(k == NH - 1),
                                )
                        nc.scalar.copy(
                            out=t_n[:, hp * 2 : hp * 2 + 2, cs], in_=zp[:, :, :cw]
                        )

                def node_b(c):
                    c0, cw = NCH[c]
                    cs = slice(c0, c0 + cw)
                    nc.vector.tensor_tensor(
                        sq_n[:, :, cs], t_n[:, :, cs], t_n[:, :, cs], AL.mult
                    )
                    var_ps = pt.tile([128, H], F32, tag="t")
                    for k in range(NH):
                        nc.tensor.matmul(
                            var_ps[:, :cw],
                            lhsT=ones_t[:],
                            rhs=sq_n[:, k, cs],
                            start=(k == 0),
                            stop=(k == NH - 1),
                        )
                    nc.scalar.activation(
                        rstd_n[:, cs],
                        var_ps[:, :cw],
                        AF.Abs_reciprocal_sqrt,
                        bias=eps_t[:, 0:1],
                        scale=1.0,
                    )
                    nc.vector.tensor_tensor(
                        m2_t[:, :, cs], t_n[:, :, cs], _bc(rstd_n[:, cs], NH, cw), AL.mult
                    )
                    nc.vector.tensor_scalar_max(m2_t[:, :, cs], m2_t[:, :, cs], 0.0)

                def node_c(c):
                    c0, cw = NCH[c]
                    cs = slice(c0, c0 + cw)
                    np_ = pm.tile([128, 2, 512], F32, tag="mm")
                    for dd in range(ND):
                        ds_ = slice(dd * 128, (dd + 1) * 128)
                        for k in range(NH):
                            nc.tensor.matmul(
                                np_[:, dd, :cw],
                                lhsT=nw3_t[:, k, ds_],
                                rhs=m2_t[:, k, cs],
                                start=(k == 0),
                                stop=(k == NH - 1),
                            )
                    nc.scalar.copy(out=node_t[:, :, cs], in_=np_[:, :, :cw])
                    for k in range(ND):
                        nc.vector.tensor_tensor(
                            flat_t[:, k, cs], flat_t[:, k, cs], node_t[:, k, cs], AL.add
                        )
                    if r < ROUNDS - 1:
                        cw_uv = min(c0 + cw, EC) - c0
                        for hp in range(2):
                            up = pm.tile([128, 2, 512], F32, tag="mm")
                            vp = pm.tile([128, 2, 512], F32, tag="mm")
                            for hh in range(2):
                                h = hp * 2 + hh
                                hs = slice(h * 128, (h + 1) * 128)
                                for k in range(ND):
                                    nc.tensor.matmul(
                                        up[:, hh, :cw_uv],
                                        lhsT=w1a_t[:, k, hs],
                                        rhs=flat_t[:, k, c0 : c0 + cw_uv],
                                        start=(k == 0),
                                        stop=False,
                                    )
                                for k in range(ND):
                                    nc.tensor.matmul(
                                        up[:, hh, :cw_uv],
                                        lhsT=w1b_t[:, k, hs],
                                        rhs=node_t[:, k, c0 : c0 + cw_uv],
                                        start=False,
                                        stop=(k == ND - 1),
                                    )
                                for k in range(ND):
                                    nc.tensor.matmul(
                                        vp[:, hh, :cw_uv],
                                        lhsT=w1c_t[:, k, hs],
                                        rhs=flat_t[:, k, c0 : c0 + cw_uv],
                                        start=(k == 0),
                                        stop=(k == ND - 1),
                                    )
                            nc.scalar.copy(
                                out=v_t[:, hp * 2 : hp * 2 + 2, c0 : c0 + cw_uv],
                                in_=vp[:, :, :cw_uv],
                            )
                            nc.scalar.copy(
                                out=u_t[:, hp * 2 : hp * 2 + 2, c0 : c0 + cw_uv],
                                in_=up[:, :, :cw_uv],
                            )

                node_a(0)
                emit_m1(1, r)
                node_a(1)
                node_b(0)
                node_b(1)
                node_c(0)
                node_c(1)

            # ---------------- the 8 rounds ----------------
            emit_uv(include_node=False)
            for r in range(ROUNDS):
                pending = None  # (slot, s_row) awaiting transpose
                for i in range(S):
                    h1 = stage_h1(i)
                    h2 = stage_t(i, h1)
                    if pending is not None:
                        stage_tr(*pending)
                    pending = (i, stage_s(i, h2))
                    if i == 6:
                        emit_m1(0, r)  # s slots 0-3 ready well before this
                stage_tr(*pending)
                emit_node(r)  # node_c also emits next round's u/v

            for k in range(ND):
                nc.sync.dma_start(yT[k], flat_t[:, k, :])

    nc.compile()
    return nc


# ---------------------------------------------------------------------------
# Host side
# ---------------------------------------------------------------------------


def _host_prep(inputs):
    """Build per-core input maps.  Returns (in_maps, perm) where perm maps
    device column order back to global flat-row order."""
    states = np.asarray(inputs["states"], np.float32).reshape(B * N, D)
    action = np.asarray(inputs["action"]).astype(np.int64)

    e_w1 = np.asarray(inputs["e_w1"], np.float32)
    e_w2 = np.asarray(inputs["e_w2"], np.float32)
    e_w3 = np.asarray(inputs["e_w3"], np.float32)
    n_w1 = np.asarray(inputs["n_w1"], np.float32)
    n_w2 = np.asarray(inputs["n_w2"], np.float32)
    n_w3 = np.asarray(inputs["n_w3"], np.float32)

    w1a, w1b, w1c = e_w1[0:D], e_w1[D : 2 * D], e_w1[2 * D : 3 * D]
    n1x = n_w1[0:D]
    n1a = n_w1[D : D + A]
    n1g = n_w1[D + A :]
    wsg = e_w3 @ n1g  # [H, H]

    # LN mean folded into centered layer-2 weights
    ew2c = e_w2 - e_w2.mean(1, keepdims=True)
    nw2c = n_w2 - n_w2.mean(1, keepdims=True)

    onehot = np.zeros((B, A), np.float32)
    onehot[np.arange(B), action] = 1.0

    # device column -> global flat row, per core
    perms = []
    for d in range(M):
        edge_rows = np.empty(EC, np.int64)
        for s in range(S):
            for b in range(NB):
                edge_rows[s * NB + b] = 896 * d + 7 * b + s
        tail_rows = np.arange(B * S + TC * d, B * S + TC * (d + 1), dtype=np.int64)
        perms.append(np.concatenate([edge_rows, tail_rows]))

    def kt(w, nk):  # [K, F] -> [nk, 128, F]
        return np.ascontiguousarray(w.reshape(nk, 128, -1))

    bf = np.float16
    common = {
        "w1a": kt(w1a, ND),
        "w1b": kt(w1b, ND),
        "w1c": kt(w1c, ND),
        "n1x": kt(n1x, ND),
        "n1a": np.ascontiguousarray(n1a).astype(bf),
        "ew2c": kt(ew2c, NH).astype(bf),
        "wsg": kt(wsg, NH).astype(bf),
        "nw2c": kt(nw2c, NH).astype(bf),
        "nw3": kt(n_w3, NH).astype(bf),
        "ident": np.eye(128, dtype=bf),
    }

    in_maps = []
    for d in range(M):
        rows = perms[d]
        xT_d = np.ascontiguousarray(states[rows].T.reshape(ND, 128, C))
        oh_d = np.ascontiguousarray(onehot[rows // N].T).astype(bf)  # [A, C]
        in_maps.append({"xT": xT_d, "ohT": oh_d, **common})
    return in_maps, perms


def _check_fast_path(inputs):
    z = lambda k: np.allclose(np.asarray(inputs[k]), 0.0)
    o = lambda k: np.allclose(np.asarray(inputs[k]), 1.0)
    return (
        z("e_b1") and z("e_b2") and z("e_bn") and z("e_b3")
        and z("n_b1") and z("n_b2") and z("n_bn") and z("n_b3")
        and o("e_g") and o("n_g")
    )


def _numpy_fallback(inputs):
    """Exact NumPy port of reference.py (used only if the fast-path
    assumptions about biases/LN-affine do not hold)."""
    f32 = np.float32
    states = np.asarray(inputs["states"], f32)
    action = np.asarray(inputs["action"]).astype(np.int64)
    g = {k: np.asarray(v, f32) for k, v in inputs.items() if k not in ("states", "action")}

    def ln(x, ga, be):
        m = x.mean(-1, keepdims=True)
        v = x.var(-1, keepdims=True)
        return (x - m) / np.sqrt(v + EPS) * ga + be

    def mlp(x, w1, b1, w2, b2, ga, bn, w3, b3):
        h = np.maximum(x @ w1 + b1, 0)
        h = np.maximum(ln(h @ w2 + b2, ga, bn), 0)
        return h @ w3 + b3

    eP = (g["e_w1"], g["e_b1"], g["e_w2"], g["e_b2"], g["e_g"], g["e_bn"], g["e_w3"], g["e_b3"])
    nP = (g["n_w1"], g["n_b1"], g["n_w2"], g["n_b2"], g["n_g"], g["n_bn"], g["n_w3"], g["n_b3"])
    flat = states.reshape(-1, D)
    pairs = np.array([(i, j) for i in range(S) for j in range(S) if i != j], np.int64)
    off = (np.arange(B, dtype=np.int64) * S)[:, None]
    row = (pairs[:, 0][None, :] + off).reshape(-1)
    col = (pairs[:, 1][None, :] + off).reshape(-1)
    E = row.shape[0]
    onehot = np.zeros((B, A), f32)
    onehot[np.arange(B), action] = 1.0
    av = np.repeat(onehot, N, axis=0)

    def seg_sum(e):
        agg = np.zeros((B * N, H), f32)
        np.add.at(agg, row, e)
        return agg

    e = mlp(np.concatenate([flat[row], np.zeros((E, D), f32), flat[col]], 1), *eP)
    node = mlp(np.concatenate([flat, av, seg_sum(e)], 1), *nP)
    flat = flat + node
    av0 = np.zeros_like(av)
    for _ in range(N - 1):
        e = mlp(np.concatenate([flat[row], node[row], flat[col]], 1), *eP)
        node = mlp(np.concatenate([flat, av0, seg_sum(e)], 1), *nP)
        flat = flat + node
    return flat.reshape(B, N, D).astype(np.float32)


def get_program():
    if "nc" not in _CACHE:
        _CACHE["nc"] = build_program()
    return _CACHE["nc"]


def kernel(**inputs):
    if not _check_fast_path(inputs):
        return _numpy_fallback(inputs)

    nc = get_program()
    in_maps, perms = _host_prep(inputs)
    res = bass_utils.run_bass_kernel_spmd(nc, in_maps, core_ids=list(range(M)))
    _CACHE["last_results"] = res

    out = np.empty((B * N, D), np.float32)
    for d in range(M):
        yT = res.results[d]["yT"].reshape(D, C)  # [D, C]
        out[perms[d]] = yT.T
    return out.reshape(B, N, D)


if __name__ == "__main__":
    rng = np.random.default_rng(0)
    print("building program...")
    nc = get_program()
    print("built.")
